# revision 13
# baseline (speedup 1.0000x reference)
"""Trainium2 Bass kernel for Cluster-Level GCN (8 NeuronCores).

Strategy:
  - Cluster-sharded segment-sum: host sorts feature rows by cluster label and
    assigns each core a contiguous range of 1024 clusters (all rows of those
    clusters). Each core computes its 1024 cluster means via one-hot matmuls
    accumulated in PSUM, then an AllGather (8MB/rank) replicates the full
    [8192, 2048] cluster-mean table.
  - Data-parallel GCN over B: each core handles 32 queries, packed 2 per
    128-partition tile. Gram matrix / softmax / both GraphConv layers /
    classifier run on-chip; algebra is reorganized as S@(X@W) so activations
    never need transposing beyond cheap PE transposes of X.
"""

import os
import sys

sys.path.insert(0, "/opt/trn_rl_repo")

import numpy as np

import concourse.bacc as bacc
import concourse.bass as bass
import concourse.tile as tile
from concourse import mybir
from concourse.bass_utils import run_bass_kernel_spmd
from concourse.masks import make_identity

F32 = mybir.dt.float32
I32 = mybir.dt.int32

N, C, B, K = 65536, 8192, 256, 64
D, NHID, DOUT, NCLASS = 2048, 512, 256, 2
NCORES = 8
CPC = C // NCORES          # clusters per core
QPC = B // NCORES          # queries per core
PACKS = QPC // 2           # 2 queries per 128-row pack
NGRP = CPC // 128          # 8 cluster groups of 128 per core

MODE = os.environ.get("KMODE", "fp32r")  # fp32 | fp32r | bf16

_CACHE = {}


def _mmdt_np():
    if MODE == "bf16":
        import ml_dtypes
        return ml_dtypes.bfloat16
    return np.float32


def _mmdt():
    if MODE == "bf16":
        return mybir.dt.bfloat16
    if MODE == "fp32r":
        return mybir.dt.float32r
    return F32


def _build(T_G: int):
    """Build the SPMD bass program. T_G = row-tiles per cluster group."""
    nc = bacc.Bacc("TRN2", target_bir_lowering=False, debug=False,
                   enable_asserts=False, num_devices=NCORES)
    MM = _mmdt()
    NT = NGRP * T_G

    TDT = MM if MODE == "bf16" else F32  # dtype for PE transposes
    FDT = MM if MODE == "fp32r" else F32  # feature-stream dtype

    def mm(out, lhsT, rhs, start, stop):
        nc.tensor.matmul(out, lhsT=lhsT, rhs=rhs, start=start, stop=stop)

    # ---- I/O ----
    feat_rows = nc.dram_tensor("feat_rows", [NT * 128, D], FDT, kind="ExternalInput").ap()
    onehot = nc.dram_tensor("onehot", [NT, 128, 128], MM, kind="ExternalInput").ap()
    scale_inv = nc.dram_tensor("scale_inv", [NGRP, 128], F32, kind="ExternalInput").ap()
    gidx = nc.dram_tensor("gidx", [PACKS, 128], I32, kind="ExternalInput").ap()
    anchors = nc.dram_tensor("anchors", [QPC, D], F32, kind="ExternalInput").ap()
    cs_all = nc.dram_tensor("cs_all", [1, PACKS * 128], F32, kind="ExternalInput").ap()
    w1_d = nc.dram_tensor("w1", [2 * D, NHID], MM, kind="ExternalInput").ap()
    w2_d = nc.dram_tensor("w2", [2 * NHID, DOUT], MM, kind="ExternalInput").ap()
    cw1_d = nc.dram_tensor("cw1", [DOUT, DOUT], MM, kind="ExternalInput").ap()
    cw2_d = nc.dram_tensor("cw2", [DOUT, NCLASS], MM, kind="ExternalInput").ap()
    b1_d = nc.dram_tensor("b1", [1, NHID], F32, kind="ExternalInput").ap()
    b2_d = nc.dram_tensor("b2", [1, DOUT], MM, kind="ExternalInput").ap()
    cb1_d = nc.dram_tensor("cb1r", [2, 128], F32, kind="ExternalInput").ap()
    alpha_d = nc.dram_tensor("alphar", [2, 128], F32, kind="ExternalInput").ap()
    cb2_d = nc.dram_tensor("cb2r", [NCLASS, 1], F32, kind="ExternalInput").ap()

    clu_out = nc.dram_tensor("clu_out", [CPC, D], F32, kind="ExternalOutput").ap()
    simm_out = nc.dram_tensor("simm_out", [CPC, 1], F32, kind="ExternalOutput").ap()
    pred_out = nc.dram_tensor("pred_out", [PACKS * 128, NCLASS], F32, kind="ExternalOutput").ap()

    with tile.TileContext(nc) as tc:
        with tc.tile_pool(name="dram", bufs=1, space="DRAM") as dram:
            cc_in = dram.tile([CPC, D], F32)
            clu_full = dram.tile([C, D], F32, addr_space="Shared")

            # ================= Phase 1: segment sum =================
            with tc.tile_pool(name="p1sb", bufs=4) as p1sb, \
                 tc.tile_pool(name="p1oh", bufs=4) as p1oh, \
                 tc.tile_pool(name="p1out", bufs=2) as p1out, \
                 tc.tile_pool(name="p1sc", bufs=2) as p1sc, \
                 tc.tile_pool(name="p1keep", bufs=1) as p1keep, \
                 tc.tile_pool(name="p1ps", bufs=2, space="PSUM") as p1ps:
                simm_sb = p1keep.tile([128, NGRP], F32)
                for g in range(NGRP):
                    psum_g = p1ps.tile([128, D], F32, name="psum_g")
                    for t in range(T_G):
                        gt = g * T_G + t
                        ft = p1sb.tile([128, D], FDT, name="ft")
                        nc.sync.dma_start(out=ft[:], in_=feat_rows[gt * 128:(gt + 1) * 128, :])
                        oh = p1oh.tile([128, 128], MM, name="oh")
                        nc.sync.dma_start(out=oh[:], in_=onehot[gt])
                        if MODE == "bf16":
                            ftm = p1sb.tile([128, D], MM, name="ftm")
                            nc.scalar.copy(ftm[:], ft[:])
                        else:
                            ftm = ft
                        for j in range(D // 512):
                            mm(psum_g[:, j * 512:(j + 1) * 512], oh[:],
                               ftm[:, j * 512:(j + 1) * 512],
                               start=(t == 0), stop=(t == T_G - 1))
                    sc = p1sc.tile([128, 1], F32, name="sc")
                    nc.sync.dma_start(out=sc[:], in_=scale_inv[g, :, None])
                    clu_sb = p1out.tile([128, D], F32, name="clu_sb")
                    nc.vector.tensor_scalar_mul(clu_sb[:], psum_g[:], sc[:, 0:1])
                    sq = p1out.tile([128, D], F32, name="sq")
                    nc.scalar.activation(sq[:], clu_sb[:],
                                         mybir.ActivationFunctionType.Square,
                                         accum_out=simm_sb[:, g:g + 1])
                    nc.sync.dma_start(out=clu_out[g * 128:(g + 1) * 128, :], in_=clu_sb[:])
                    nc.sync.dma_start(out=cc_in[g * 128:(g + 1) * 128, :], in_=clu_sb[:])
                for g in range(NGRP):
                    nc.sync.dma_start(out=simm_out[g * 128:(g + 1) * 128, :],
                                      in_=simm_sb[:, g:g + 1])

            # ================= AllGather =================
            nc.gpsimd.collective_compute(
                "AllGather", mybir.AluOpType.bypass,
                replica_groups=[list(range(NCORES))],
                ins=[cc_in[:]], outs=[clu_full[:]],
            )

            # ================= Phase 2: GCN =================
            with tc.tile_pool(name="wpool", bufs=1) as wp, \
                 tc.tile_pool(name="xp", bufs=3) as xp, \
                 tc.tile_pool(name="scr", bufs=2) as scrp, \
                 tc.tile_pool(name="xtp", bufs=2) as xtp, \
                 tc.tile_pool(name="sm", bufs=3) as smp, \
                 tc.tile_pool(name="mid", bufs=2) as midp, \
                 tc.tile_pool(name="tiny", bufs=4) as tyP, \
                 tc.tile_pool(name="psT", bufs=2, space="PSUM") as psT, \
                 tc.tile_pool(name="psB", bufs=5, space="PSUM") as psB:

                # weights / constants (resident)
                w1_sb = wp.tile([128, 2 * D // 128, NHID], MM)
                nc.sync.dma_start(out=w1_sb[:], in_=w1_d.rearrange("(c p) n -> p c n", p=128))
                w2_sb = wp.tile([128, 2 * NHID // 128, DOUT], MM)
                nc.sync.dma_start(out=w2_sb[:], in_=w2_d.rearrange("(c p) n -> p c n", p=128))
                cw1_sb = wp.tile([128, DOUT // 128, DOUT], MM)
                nc.sync.dma_start(out=cw1_sb[:], in_=cw1_d.rearrange("(c p) n -> p c n", p=128))
                cw2_sb = wp.tile([128, DOUT // 128, NCLASS], MM)
                nc.sync.dma_start(out=cw2_sb[:], in_=cw2_d.rearrange("(c p) n -> p c n", p=128))
                b1_sb = wp.tile([128, NHID], F32)
                nc.sync.dma_start(out=b1_sb[0:1, :], in_=b1_d[:])
                nc.sync.dma_start(out=b1_sb[64:65, :], in_=b1_d[:])
                b2_sb = wp.tile([1, DOUT], MM)
                nc.sync.dma_start(out=b2_sb[:], in_=b2_d[:])
                cb1_sb = wp.tile([128, 2], F32)
                alpha_sb = wp.tile([128, 2], F32)
                for m in range(2):
                    nc.sync.dma_start(out=cb1_sb[:, m:m + 1], in_=cb1_d[m, :, None])
                    nc.sync.dma_start(out=alpha_sb[:, m:m + 1], in_=alpha_d[m, :, None])
                cb2_sb = wp.tile([NCLASS, 1], F32)
                nc.sync.dma_start(out=cb2_sb[:], in_=cb2_d[:])

                id_f32 = wp.tile([128, 128], F32)
                make_identity(nc, id_f32[:])
                if MODE == "bf16":
                    id_mm = wp.tile([128, 128], MM)
                    nc.vector.tensor_copy(id_mm[:], id_f32[:])
                else:
                    id_mm = id_f32

                ones_f = wp.tile([1, 128], F32)
                nc.vector.memset(ones_f[:], 1.0)
                ind_f = wp.tile([128, 128], F32)
                nc.vector.memset(ind_f[:], 0.0)
                nc.vector.memset(ind_f[0:1, 0:64], 1.0)
                nc.vector.memset(ind_f[64:65, 64:128], 1.0)
                if MM == F32:
                    ones_sb, ind_sb = ones_f, ind_f
                else:
                    ones_sb = wp.tile([1, 128], MM)
                    nc.vector.tensor_copy(ones_sb[:], ones_f[:])
                    ind_sb = wp.tile([128, 128], MM)
                    nc.vector.tensor_copy(ind_sb[:], ind_f[:])
                amask = wp.tile([128, 128], F32)
                nc.vector.memset(amask[:], 0.0)
                nc.vector.memset(amask[0:64, 64:128], -1e30)
                nc.vector.memset(amask[64:128, 0:64], -1e30)

                DCH = D // 128  # 16 feature chunks

                for p in range(PACKS):
                    idx_t = tyP.tile([128, 1], I32, name="idx_t")
                    nc.sync.dma_start(out=idx_t[:], in_=gidx[p, :, None])
                    X2 = xp.tile([128, D], F32, name="X2")
                    nc.gpsimd.indirect_dma_start(
                        out=X2[:], out_offset=None, in_=clu_full[:],
                        in_offset=bass.IndirectOffsetOnAxis(ap=idx_t[:, 0:1], axis=0))
                    nc.sync.dma_start(out=X2[0:1, :], in_=anchors[2 * p:2 * p + 1, :])
                    nc.sync.dma_start(out=X2[64:65, :], in_=anchors[2 * p + 1:2 * p + 2, :])

                    # row norms of raw X (with anchors)
                    sq2 = scrp.tile([128, D], F32, name="sq2")
                    ss = tyP.tile([128, 1], F32, name="ss")
                    nc.scalar.activation(sq2[:], X2[:],
                                         mybir.ActivationFunctionType.Square,
                                         accum_out=ss[:])
                    nrm = tyP.tile([128, 1], F32, name="nrm")
                    nc.scalar.sqrt(nrm[:], ss[:])
                    rn_inv = tyP.tile([128, 1], F32, name="rn_inv")
                    nc.vector.reciprocal(rn_inv[:], nrm[:])

                    # transpose X2 -> X2T [128, DCH, 128] (in MM dtype)
                    if MODE == "bf16":
                        xin = xp.tile([128, D], MM, name="xin")
                        nc.vector.tensor_copy(xin[:], X2[:])
                    else:
                        xin = X2
                    X2T = xtp.tile([128, DCH, 128], MM, name="X2T")
                    for c in range(DCH):
                        pst = psT.tile([128, 128], TDT, space="PSUM", name="pst", tag="pst")
                        nc.tensor.transpose(pst[:], xin[:, c * 128:(c + 1) * 128], id_mm[:])
                        nc.vector.tensor_copy(X2T[:, c, :], pst[:])

                    # gram A2 = X2 @ X2^T
                    psA = psB.tile([128, 512], F32, space="PSUM", name="psA", tag="psb")
                    for c in range(DCH):
                        mm(psA[:, 0:128], X2T[:, c, :], X2T[:, c, :],
                           start=(c == 0), stop=(c == DCH - 1))

                    # CS2 = broadcast cs row; s = A2*CS2 + mask
                    cs_row = tyP.tile([1, 128], F32, name="cs_row")
                    nc.sync.dma_start(out=cs_row[:], in_=cs_all[:, p * 128:(p + 1) * 128])
                    CS2 = smp.tile([128, 128], F32, name="CS2")
                    nc.gpsimd.partition_broadcast(CS2[:], cs_row[0:1, :])
                    s_sb = smp.tile([128, 128], F32, name="s_sb")
                    nc.vector.tensor_tensor(out=s_sb[:], in0=psA[:, 0:128], in1=CS2[:],
                                            op=mybir.AluOpType.mult)
                    nc.vector.tensor_tensor(out=s_sb[:], in0=s_sb[:], in1=amask[:],
                                            op=mybir.AluOpType.add)
                    negmax = tyP.tile([128, 1], F32, name="negmax")
                    nc.vector.reduce_max(negmax[:], s_sb[:], axis=mybir.AxisListType.X,
                                         negate=True)
                    e_sb = smp.tile([128, 128], F32, name="e_sb")
                    sume = tyP.tile([128, 1], F32, name="sume")
                    nc.scalar.activation(e_sb[:], s_sb[:],
                                         mybir.ActivationFunctionType.Exp,
                                         bias=negmax[:, 0:1], accum_out=sume[:])
                    rsum = tyP.tile([128, 1], F32, name="rsum")
                    nc.vector.reciprocal(rsum[:], sume[:])
                    S2 = smp.tile([128, 128], TDT, name="S2")
                    nc.vector.tensor_scalar_mul(S2[:], e_sb[:], rsum[:, 0:1])
                    psS = psT.tile([128, 128], TDT, space="PSUM", name="psS", tag="pst")
                    nc.tensor.transpose(psS[:], S2[:], id_mm[:])
                    S2T = smp.tile([128, 128], MM, name="S2T")
                    nc.vector.tensor_copy(S2T[:], psS[:])

                    # ---- layer 1 ----
                    ps1 = psB.tile([128, 512], F32, space="PSUM", name="ps1", tag="psb")
                    for c in range(DCH):
                        mm(ps1[:], X2T[:, c, :], w1_sb[:, c, :],
                           start=(c == 0), stop=(c == DCH - 1))
                    ps2 = psB.tile([128, 512], F32, space="PSUM", name="ps2", tag="psb")
                    for c in range(DCH):
                        mm(ps2[:], X2T[:, c, :], w1_sb[:, DCH + c, :],
                           start=(c == 0), stop=(c == DCH - 1))
                    hA = midp.tile([128, NHID], F32, name="hA")
                    nc.vector.tensor_scalar_mul(hA[:], ps1[:], rn_inv[:, 0:1])
                    hB = midp.tile([128, NHID], MM, name="hB")
                    nc.vector.tensor_scalar_mul(hB[:], ps2[:], rn_inv[:, 0:1])
                    # vneg rows: b1 - hA[r] - hB[r] for r in {0, 64} (built in-place)
                    vn = midp.tile([128, NHID], MM, name="vn")
                    for r in (0, 64):
                        nc.vector.tensor_scalar_mul(vn[r:r + 1, :], ps2[r:r + 1, :],
                                                    rn_inv[r:r + 1, 0:1])
                        nc.vector.tensor_tensor(out=vn[r:r + 1, :], in0=vn[r:r + 1, :],
                                                in1=hA[r:r + 1, :], op=mybir.AluOpType.add)
                        nc.vector.tensor_tensor(out=vn[r:r + 1, :], in0=b1_sb[r:r + 1, :],
                                                in1=vn[r:r + 1, :], op=mybir.AluOpType.subtract)
                    ps3 = psB.tile([128, 512], F32, space="PSUM", name="ps3", tag="psb")
                    mm(ps3[:], S2T[:], hB[:], start=True, stop=False)
                    mm(ps3[:], ind_sb[0:1, :], vn[0:1, :], start=False, stop=False)
                    mm(ps3[:], ind_sb[64:65, :], vn[64:65, :], start=False, stop=True)
                    t1 = midp.tile([128, NHID], F32, name="t1")
                    nc.vector.tensor_tensor(out=t1[:], in0=hA[:], in1=ps3[:],
                                            op=mybir.AluOpType.add)
                    out1 = midp.tile([128, NHID], TDT, name="out1")
                    nc.scalar.activation(out1[:], t1[:], mybir.ActivationFunctionType.Relu)

                    # out1T
                    out1T = midp.tile([128, NHID // 128, 128], MM, name="out1T")
                    for c in range(NHID // 128):
                        pst2 = psT.tile([128, 128], TDT, space="PSUM", name="pst", tag="pst")
                        nc.tensor.transpose(pst2[:], out1[:, c * 128:(c + 1) * 128], id_mm[:])
                        nc.vector.tensor_copy(out1T[:, c, :], pst2[:])

                    # ---- layer 2 ----
                    ps4 = psB.tile([128, 512], F32, space="PSUM", name="ps4", tag="psb")
                    for c in range(NHID // 128):
                        mm(ps4[:, 0:DOUT], out1T[:, c, :], w2_sb[:, 4 + c, :],
                           start=(c == 0), stop=(c == NHID // 128 - 1))
                    h2b = midp.tile([128, DOUT], MM, name="h2b")
                    nc.vector.tensor_copy(h2b[:], ps4[:, 0:DOUT])
                    ps5 = psB.tile([128, 512], F32, space="PSUM", name="ps5", tag="psb")
                    for c in range(NHID // 128):
                        mm(ps5[:, 0:DOUT], out1T[:, c, :], w2_sb[:, c, :],
                           start=(c == 0), stop=False)
                    mm(ps5[:, 0:DOUT], S2T[:], h2b[:], start=False, stop=False)
                    mm(ps5[:, 0:DOUT], ones_sb[:], b2_sb[:], start=False, stop=True)
                    out2 = midp.tile([128, DOUT], TDT, name="out2")
                    nc.scalar.activation(out2[:], ps5[:, 0:DOUT],
                                         mybir.ActivationFunctionType.Relu)

                    out2T = midp.tile([128, DOUT // 128, 128], MM, name="out2T")
                    for c in range(DOUT // 128):
                        pst3 = psT.tile([128, 128], TDT, space="PSUM", name="pst", tag="pst")
                        nc.tensor.transpose(pst3[:], out2[:, c * 128:(c + 1) * 128], id_mm[:])
                        nc.vector.tensor_copy(out2T[:, c, :], pst3[:])

                    # ---- classifier ----
                    h3p = midp.tile([128, DOUT // 128, 128], MM, name="h3p")
                    for m2 in range(DOUT // 128):
                        ps6 = psB.tile([128, 512], F32, space="PSUM", name="ps6", tag="psb")
                        for c in range(DOUT // 128):
                            mm(ps6[:, 0:128], cw1_sb[:, c, m2 * 128:(m2 + 1) * 128],
                               out2T[:, c, :], start=(c == 0), stop=(c == DOUT // 128 - 1))
                        nc.scalar.activation(h3p[:, m2, :], ps6[:, 0:128],
                                             mybir.ActivationFunctionType.Prelu,
                                             bias=cb1_sb[:, m2:m2 + 1],
                                             alpha=alpha_sb[:, m2:m2 + 1])
                    ps7 = psT.tile([128, 128], F32, space="PSUM", name="ps7f", tag="pst")
                    for m2 in range(DOUT // 128):
                        mm(ps7[0:NCLASS, 0:128], cw2_sb[:, m2, :], h3p[:, m2, :],
                           start=(m2 == 0), stop=(m2 == DOUT // 128 - 1))
                    lt = tyP.tile([NCLASS, 128], F32, name="lt")
                    nc.vector.tensor_scalar_add(lt[:], ps7[0:NCLASS, 0:128], cb2_sb[:, 0:1])
                    psL = psT.tile([128, 128], F32, space="PSUM", name="psLf", tag="pst")
                    nc.tensor.transpose(psL[0:128, 0:NCLASS], lt[:], id_f32[0:NCLASS, 0:NCLASS])
                    # softmax over the 2 classes
                    lmax = tyP.tile([128, 1], F32, name="lmax")
                    nc.vector.reduce_max(lmax[:], psL[0:128, 0:NCLASS],
                                         axis=mybir.AxisListType.X, negate=True)
                    epr = tyP.tile([128, NCLASS], F32, name="epr")
                    sepr = tyP.tile([128, 1], F32, name="sepr")
                    nc.scalar.activation(epr[:], psL[0:128, 0:NCLASS],
                                         mybir.ActivationFunctionType.Exp,
                                         bias=lmax[:, 0:1], accum_out=sepr[:])
                    rpr = tyP.tile([128, 1], F32, name="rpr")
                    nc.vector.reciprocal(rpr[:], sepr[:])
                    prt = tyP.tile([128, NCLASS], F32, name="prt")
                    nc.vector.tensor_scalar_mul(prt[:], epr[:], rpr[:, 0:1])
                    nc.sync.dma_start(out=pred_out[p * 128:(p + 1) * 128, :], in_=prt[:])

    nc.compile()
    return nc


def _host_prep(inputs):
    """Shard + preprocess on host. Returns (T_G, in_maps, nums, counts)."""
    feats = np.ascontiguousarray(np.asarray(inputs["features"], dtype=np.float32))
    labels = np.asarray(inputs["labels"]).astype(np.int64)
    indexes = np.asarray(inputs["indexes"]).astype(np.int64)
    knn = np.asarray(inputs["ori_knn_neighbor"]).astype(np.int64)
    all_pred = np.asarray(inputs["all_pred"], dtype=np.float32)

    mmnp = _mmdt_np()
    counts = np.bincount(labels, minlength=C)
    order = np.argsort(labels, kind="stable")
    slab = labels[order]
    bounds = np.searchsorted(slab, np.arange(0, C + 1, 128))
    glens = bounds[1:] - bounds[:-1]          # rows per 128-cluster group (64 groups)
    T_G = int(np.ceil(glens.max() / 128)) if glens.max() > 0 else 1
    NT = NGRP * T_G

    scale_all = (1.0 / np.maximum(counts, 1)).astype(np.float32)
    cs_full = np.exp(all_pred[:, :, 1]).astype(np.float32)      # [B, K]
    clu_lab = labels[knn].astype(np.int32)                      # [B, K]

    w1 = np.asarray(inputs["w1"], np.float32).astype(mmnp)
    w2 = np.asarray(inputs["w2"], np.float32).astype(mmnp)
    cw1 = np.asarray(inputs["cw1"], np.float32).astype(mmnp)
    cw2 = np.asarray(inputs["cw2"], np.float32).astype(mmnp)
    b1 = np.asarray(inputs["b1"], np.float32).reshape(1, NHID)
    b2 = np.asarray(inputs["b2"], np.float32).reshape(1, DOUT)
    cb1r = np.asarray(inputs["cb1"], np.float32).reshape(2, 128)
    alphar = np.asarray(inputs["alpha"], np.float32).reshape(2, 128)
    cb2r = np.asarray(inputs["cb2"], np.float32).reshape(NCLASS, 1)

    in_maps = []
    for c in range(NCORES):
        feat_rows = np.zeros((NT * 128, D), np.float32)
        onehot = np.zeros((NT, 128, 128), mmnp)
        for g in range(NGRP):
            gg = c * NGRP + g
            lo, hi = bounds[gg], bounds[gg + 1]
            n = hi - lo
            if n > 0:
                rows = order[lo:hi]
                base = g * T_G * 128
                feat_rows[base:base + n] = feats[rows]
                tl = np.arange(n)
                cols = (slab[lo:hi] - 128 * gg).astype(np.int64)
                onehot[g * T_G + tl // 128, tl % 128, cols] = 1.0
        scale_c = scale_all[c * CPC:(c + 1) * CPC].reshape(NGRP, 128)
        q0 = c * QPC
        gq = clu_lab[q0:q0 + QPC, :].copy()
        gq[:, 0] = 0  # slot 0 is overwritten by the anchor row on-device
        gidx = gq.reshape(PACKS, 128).astype(np.int32)
        anchors = feats[indexes[q0:q0 + QPC]]
        cs_all_c = cs_full[q0:q0 + QPC, :].reshape(1, PACKS * 128)
        in_maps.append({
            "feat_rows": feat_rows, "onehot": onehot, "scale_inv": scale_c,
            "gidx": gidx, "anchors": anchors, "cs_all": cs_all_c,
            "w1": w1, "w2": w2, "cw1": cw1, "cw2": cw2,
            "b1": b1, "b2": b2, "cb1r": cb1r, "alphar": alphar, "cb2r": cb2r,
        })
    nums = counts.astype(np.float32).reshape(C, 1)
    return T_G, in_maps, nums


def kernel(**inputs):
    T_G, in_maps, nums = _host_prep(inputs)
    key = (T_G, MODE)
    if key not in _CACHE:
        _CACHE[key] = _build(T_G)
    nc = _CACHE[key]
    import time as _time
    _t0 = _time.time()
    try:
        res = run_bass_kernel_spmd(nc, in_maps, core_ids=list(range(NCORES)),
                                   trace=bool(int(os.environ.get("KTRACE", "0"))))
    except ModuleNotFoundError:
        # NTFF profiling hook unavailable under this axon client; run untraced.
        res = run_bass_kernel_spmd(nc, in_maps, core_ids=list(range(NCORES)))
    kernel.last_spmd_seconds = _time.time() - _t0
    kernel.last_results = res
    pred = np.concatenate([r["pred_out"].reshape(QPC, K, NCLASS) for r in res.results])
    simm = np.concatenate([r["simm_out"].reshape(CPC) for r in res.results])
    clu = np.concatenate([r["clu_out"] for r in res.results])
    return pred, simm, clu, nums
